# revision 2
# baseline (speedup 1.0000x reference)
"""Trainium2 Bass kernel for nn_AffConv (gnn_message_passing).

Math (per graph): out = relu(concat_k[(l[idx_k]-l)/11, f[idx_k]] ++ f) @ W + b.
The clip(-1,1) in the reference is a no-op (locs are in [0,1), so diffs/11 are
in (-0.091, 0.091)), which makes the locs contribution linear. The whole
per-node row is then a sum of 10 per-block matmuls over gathered "tokens"
(64 feat ch + 2 loc ch + pad, fp16), so the kernel is:

  - preload a per-graph token table into SBUF (channel-padded to 128, fp16)
  - per 512-node tile: SBUF-source transposed dma_gather of 10x512 tokens
    (9 kNN blocks + the center block) -> channel-major (128, 5120) tile
  - 20 accumulating PE matmuls (10 blocks x 2 table banks) into PSUM(64,512)
  - relu+bias on the scalar engine, DMA out (outC-major scratch, transposed
    on the host during unshard)

dma_gather indices are signed int16, so the 50000-token space is split into
bank A (tokens 0..32766 + a zero token at 32767) and bank B (tokens
32767..49999 local-reindexed + a zero token). Every gather position fetches a
real token from its own bank and the all-zeros token from the other bank;
PSUM accumulation of both banks' matmuls with shared weights yields the
correct sum (zero tokens contribute nothing).

Sharding: 8 cores = 4 graphs x 2 node-halves. Each core computes 25000 output
nodes of one graph and holds that graph's full token table in SBUF.
"""

import numpy as np

# problem constants (hardcoded; harness provides full inputs)
N_GRAPHS = 4
M = 50000
KNN = 9
C = 64
OUTC = 64
DIST = 10.0

P = 128
ELEM = 128              # fp16 elems per token (256B)
NT = 512                # nodes per tile
HALF = 25000            # nodes per core
NPAD = 25088            # padded to 49 tiles
TILES = NPAD // NT      # 49
NBLK = 10               # 9 kNN blocks + center block
NIDX = NBLK * NT        # gather positions per tile (5120)

A_REAL = 32767          # bank A real tokens: graph nodes [0, 32767)
A_ZERO = 32767          # bank A zero token id
A_RANKS = 32768 // P    # 256
B_REAL = M - A_REAL     # 17233 nodes [32767, 50000)
B_ZERO = B_REAL         # 17233
B_RANKS = -(-(B_REAL + 1) // P)   # 135
B_TOK = B_RANKS * P     # 17280
TAB_FREE = (A_RANKS + B_RANKS) * ELEM   # 50048 fp16 elems per partition

_module_cache = {}


def _build_module():
    import concourse.bacc as bacc
    import concourse.mybir as mybir
    import concourse.tile as tile

    nc = bacc.Bacc(None, target_bir_lowering=False, debug=False)

    tab_d = nc.dram_tensor("tab", [P, TAB_FREE], mybir.dt.float16, kind="ExternalInput")
    idx_d = nc.dram_tensor("idx", [TILES, P, 2 * NIDX // 16], mybir.dt.int16, kind="ExternalInput")
    w_d = nc.dram_tensor("w", [P, NBLK * OUTC], mybir.dt.float16, kind="ExternalInput")
    b_d = nc.dram_tensor("b", [OUTC, 1], mybir.dt.float32, kind="ExternalInput")
    out_d = nc.dram_tensor("out", [OUTC, NPAD], mybir.dt.float32, kind="ExternalOutput")

    ICOLS = NIDX // 16  # 320 idx columns per bank

    with tile.TileContext(nc) as tc:
        with (
            tc.tile_pool(name="tabp", bufs=1) as tabp,
            tc.tile_pool(name="misc", bufs=1) as misc,
            tc.tile_pool(name="idxp", bufs=3) as idxp,
            tc.tile_pool(name="gath", bufs=2) as gath,
            tc.tile_pool(name="outp", bufs=3) as outp,
            tc.tile_pool(name="psum", bufs=2, space="PSUM") as psump,
        ):
            tab_t = tabp.tile([P, TAB_FREE], mybir.dt.float16)
            nc.sync.dma_start(out=tab_t[:], in_=tab_d[:])

            w_t = misc.tile([P, NBLK * OUTC], mybir.dt.float16, tag="w")
            nc.sync.dma_start(out=w_t[:], in_=w_d[:])
            b_t = misc.tile([OUTC, 1], mybir.dt.float32, tag="b")
            nc.sync.dma_start(out=b_t[:], in_=b_d[:])

            for t in range(TILES):
                idx_t = idxp.tile([P, 2 * ICOLS], mybir.dt.int16, tag="idx")
                nc.sync.dma_start(out=idx_t[:], in_=idx_d[t])

                gA_t = gath.tile([P, NIDX], mybir.dt.float16, tag="ga")
                gB_t = gath.tile([P, NIDX], mybir.dt.float16, tag="gb")

                nc.gpsimd.dma_gather(
                    out_ap=gA_t[:].rearrange("p (o n) -> p o n", o=1),
                    in_ap=tab_t[:, 0 : A_RANKS * ELEM],
                    idxs_ap=idx_t[:, 0:ICOLS],
                    num_idxs=NIDX,
                    num_idxs_reg=NIDX,
                    elem_size=ELEM,
                    transpose=True,
                    sbuf_tokens_per_rank=P,
                    sbuf_free_dim_per_rank=ELEM * 2,
                    single_packet=False,
                )
                nc.gpsimd.dma_gather(
                    out_ap=gB_t[:].rearrange("p (o n) -> p o n", o=1),
                    in_ap=tab_t[:, A_RANKS * ELEM :],
                    idxs_ap=idx_t[:, ICOLS : 2 * ICOLS],
                    num_idxs=NIDX,
                    num_idxs_reg=NIDX,
                    elem_size=ELEM,
                    transpose=True,
                    sbuf_tokens_per_rank=P,
                    sbuf_free_dim_per_rank=ELEM * 2,
                    single_packet=False,
                )

                ps = psump.tile([OUTC, NT], mybir.dt.float32)
                for bi, g_t in enumerate((gA_t, gB_t)):
                    for k in range(NBLK):
                        nc.tensor.matmul(
                            out=ps[:],
                            lhsT=w_t[:, k * OUTC : (k + 1) * OUTC],
                            rhs=g_t[:, k * NT : (k + 1) * NT],
                            start=(bi == 0 and k == 0),
                            stop=(bi == 1 and k == NBLK - 1),
                        )

                o_t = outp.tile([OUTC, NT], mybir.dt.float32, tag="o")
                nc.scalar.activation(
                    o_t[:], ps[:], mybir.ActivationFunctionType.Relu, bias=b_t[:]
                )
                nc.sync.dma_start(out=out_d[:, t * NT : (t + 1) * NT], in_=o_t[:])

    nc.compile()
    return nc


def _swizzle_table(tok):
    """(ranks*128, ELEM) token array -> (128, ranks*ELEM) SBUF preload layout."""
    ranks = tok.shape[0] // P
    return tok.reshape(ranks, P, ELEM).transpose(1, 0, 2).reshape(P, ranks * ELEM)


def _idx_swizzle(ix):
    """(TILES, NIDX) -> (TILES, 128, NIDX//16): [t, p, s] = ix[t, s*16 + p%16]."""
    a = ix.reshape(TILES, NIDX // 16, 16).transpose(0, 2, 1)  # (T, 16, cols)
    return np.broadcast_to(a[:, None], (TILES, 8, 16, NIDX // 16)).reshape(
        TILES, P, NIDX // 16
    )


def kernel(feats, aff_idx, locs, W, b):
    from concourse.bass_utils import run_bass_kernel_spmd

    if "nc" not in _module_cache:
        _module_cache["nc"] = _build_module()
    nc = _module_cache["nc"]

    feats = np.asarray(feats)
    aff_idx = np.asarray(aff_idx)
    locs = np.asarray(locs)
    W = np.asarray(W, dtype=np.float32)
    b = np.asarray(b, dtype=np.float32)

    # per-graph token tables (shared by the two cores of each graph)
    tables = []
    for g in range(N_GRAPHS):
        tokA = np.zeros((A_RANKS * P, ELEM), np.float16)
        tokA[:A_REAL, :C] = feats[g, :A_REAL]
        tokA[:A_REAL, C : C + 2] = locs[g, :A_REAL]
        tokB = np.zeros((B_TOK, ELEM), np.float16)
        tokB[:B_REAL, :C] = feats[g, A_REAL:]
        tokB[:B_REAL, C : C + 2] = locs[g, A_REAL:]
        tables.append(
            np.concatenate([_swizzle_table(tokA), _swizzle_table(tokB)], axis=1)
        )

    # weight blocks: lhsT (128, 64) per block; rows 0:64 feat W, 64:66 locs W/11
    Wp = np.zeros((P, NBLK * OUTC), np.float32)
    wloc_sum = np.zeros((2, OUTC), np.float32)
    for k in range(KNN):
        base = k * (C + 2)
        Wp[0:C, k * OUTC : (k + 1) * OUTC] = W[base + 2 : base + 2 + C]
        Wp[C : C + 2, k * OUTC : (k + 1) * OUTC] = W[base : base + 2] / (DIST + 1.0)
        wloc_sum += W[base : base + 2]
    Wp[0:C, KNN * OUTC :] = W[KNN * (C + 2) :]
    Wp[C : C + 2, KNN * OUTC :] = -wloc_sum / (DIST + 1.0)
    Wp = Wp.astype(np.float16)

    b_in = b.reshape(OUTC, 1).astype(np.float32)

    # per-core gather indices
    in_maps = []
    for core in range(8):
        g, h = core // 2, core % 2
        m0 = h * HALF
        nodes = np.concatenate(
            [np.arange(m0, m0 + HALF), np.zeros(NPAD - HALF, np.int64)]
        )
        nbr = aff_idx[g][nodes]                      # (NPAD, 9)
        alli = np.concatenate([nbr, nodes[:, None]], axis=1)   # (NPAD, 10)
        flat = alli.reshape(TILES, NT, NBLK).transpose(0, 2, 1).reshape(TILES, NIDX)
        idxA = np.where(flat < A_REAL, flat, A_ZERO).astype(np.int16)
        idxB = np.where(flat >= A_REAL, flat - A_REAL, B_ZERO).astype(np.int16)
        idx_both = np.ascontiguousarray(
            np.concatenate([_idx_swizzle(idxA), _idx_swizzle(idxB)], axis=2)
        )
        in_maps.append(
            {"tab": tables[g], "idx": idx_both, "w": Wp, "b": b_in}
        )

    res = run_bass_kernel_spmd(nc, in_maps, core_ids=list(range(8)))
    _module_cache["last_results"] = res

    out = np.empty((N_GRAPHS, M, OUTC), np.float32)
    for core in range(8):
        g, h = core // 2, core % 2
        out[g, h * HALF : (h + 1) * HALF] = res.results[core]["out"][:, :HALF].T
    return out


# revision 3
# speedup vs baseline: 1.1378x; 1.1378x over previous
"""Trainium2 Bass kernel for nn_AffConv (gnn_message_passing).

Math (per graph): out = relu(concat_k[(l[idx_k]-l)/11, f[idx_k]] ++ f) @ W + b.
The clip(-1,1) in the reference is a no-op (locs are in [0,1), so diffs/11 are
in (-0.091, 0.091)), which makes the locs contribution linear. The whole
per-node row is then a sum of 10 per-block matmuls over gathered "tokens"
(64 feat ch + 2 loc ch + pad, fp16):

  - preload a per-graph token table into SBUF (channel-padded to 128, fp16)
  - per 512-node tile: SBUF-source transposed dma_gather of 9x512 kNN tokens
    -> channel-major (128, 4608) tile; the center block comes from a
    host-pre-transposed channel-major table via a plain strided DMA
  - 19 accumulating PE matmuls (9 kNN blocks x 2 table banks + center)
    into PSUM(64,512)
  - relu+bias on the scalar engine, DMA out (outC-major scratch, transposed
    on the host during unshard)

dma_gather indices are signed int16, so the 50000-token space is split into
bank A (tokens 0..32766 + a zero token at 32767) and bank B (tokens
32767..49999 local-reindexed + a zero token). Every gather position fetches a
real token from its own bank and the all-zeros token from the other bank;
PSUM accumulation of both banks' matmuls with shared weights yields the
correct sum (zero tokens contribute nothing).

Sharding: 8 cores = 4 graphs x 2 node-halves. Each core computes 25000 output
nodes of one graph and holds that graph's full token table in SBUF.
"""

import numpy as np

# problem constants (hardcoded; harness provides full inputs)
N_GRAPHS = 4
M = 50000
KNN = 9
C = 64
OUTC = 64
DIST = 10.0

P = 128
ELEM = 128              # fp16 elems per token (256B)
NT = 512                # nodes per tile
HALF = 25000            # nodes per core
NPAD = 25088            # padded to 49 tiles
TILES = NPAD // NT      # 49
NIDX = KNN * NT         # gather positions per tile (4608)
ICOLS = NIDX // 16      # 288 idx columns per bank

A_REAL = 32767          # bank A real tokens: graph nodes [0, 32767)
A_ZERO = 32767          # bank A zero token id
A_RANKS = 32768 // P    # 256
B_REAL = M - A_REAL     # 17233 nodes [32767, 50000)
B_ZERO = B_REAL         # 17233
B_RANKS = -(-(B_REAL + 1) // P)   # 135
B_TOK = B_RANKS * P     # 17280
TAB_FREE = (A_RANKS + B_RANKS) * ELEM   # 50048 fp16 elems per partition

_module_cache = {}


def _build_module(rep=1):
    import concourse.bacc as bacc
    import concourse.mybir as mybir
    import concourse.tile as tile

    nc = bacc.Bacc(None, target_bir_lowering=False, debug=False)

    tab_d = nc.dram_tensor("tab", [P, TAB_FREE], mybir.dt.float16, kind="ExternalInput")
    ctr_d = nc.dram_tensor("ctr", [P, NPAD], mybir.dt.float16, kind="ExternalInput")
    idx_d = nc.dram_tensor("idx", [TILES, P, 2 * ICOLS], mybir.dt.int16, kind="ExternalInput")
    w_d = nc.dram_tensor("w", [P, 10 * OUTC], mybir.dt.float16, kind="ExternalInput")
    b_d = nc.dram_tensor("b", [OUTC, 1], mybir.dt.float32, kind="ExternalInput")
    out_d = nc.dram_tensor("out", [OUTC, NPAD], mybir.dt.float32, kind="ExternalOutput")

    with tile.TileContext(nc) as tc:
        with (
            tc.tile_pool(name="tabp", bufs=1) as tabp,
            tc.tile_pool(name="misc", bufs=1) as misc,
            tc.tile_pool(name="idxp", bufs=3) as idxp,
            tc.tile_pool(name="gath", bufs=2) as gath,
            tc.tile_pool(name="ctrp", bufs=3) as ctrp,
            tc.tile_pool(name="outp", bufs=3) as outp,
            tc.tile_pool(name="psum", bufs=2, space="PSUM") as psump,
        ):
            tab_t = tabp.tile([P, TAB_FREE], mybir.dt.float16)
            nc.sync.dma_start(out=tab_t[:], in_=tab_d[:])

            w_t = misc.tile([P, 10 * OUTC], mybir.dt.float16, tag="w")
            nc.sync.dma_start(out=w_t[:], in_=w_d[:])
            b_t = misc.tile([OUTC, 1], mybir.dt.float32, tag="b")
            nc.sync.dma_start(out=b_t[:], in_=b_d[:])

            for r in range(rep):
                for t in range(TILES):
                    idx_t = idxp.tile([P, 2 * ICOLS], mybir.dt.int16, tag="idx")
                    nc.sync.dma_start(out=idx_t[:], in_=idx_d[t])

                    ctr_t = ctrp.tile([P, NT], mybir.dt.float16, tag="ctr")
                    nc.sync.dma_start(out=ctr_t[:], in_=ctr_d[:, t * NT : (t + 1) * NT])

                    gA_t = gath.tile([P, NIDX], mybir.dt.float16, tag="ga")
                    gB_t = gath.tile([P, NIDX], mybir.dt.float16, tag="gb")

                    nc.gpsimd.dma_gather(
                        out_ap=gA_t[:].rearrange("p (o n) -> p o n", o=1),
                        in_ap=tab_t[:, 0 : A_RANKS * ELEM],
                        idxs_ap=idx_t[:, 0:ICOLS],
                        num_idxs=NIDX,
                        num_idxs_reg=NIDX,
                        elem_size=ELEM,
                        transpose=True,
                        sbuf_tokens_per_rank=P,
                        sbuf_free_dim_per_rank=ELEM * 2,
                        single_packet=False,
                    )
                    nc.gpsimd.dma_gather(
                        out_ap=gB_t[:].rearrange("p (o n) -> p o n", o=1),
                        in_ap=tab_t[:, A_RANKS * ELEM :],
                        idxs_ap=idx_t[:, ICOLS : 2 * ICOLS],
                        num_idxs=NIDX,
                        num_idxs_reg=NIDX,
                        elem_size=ELEM,
                        transpose=True,
                        sbuf_tokens_per_rank=P,
                        sbuf_free_dim_per_rank=ELEM * 2,
                        single_packet=False,
                    )

                    ps = psump.tile([OUTC, NT], mybir.dt.float32)
                    nc.tensor.matmul(
                        out=ps[:],
                        lhsT=w_t[:, 9 * OUTC : 10 * OUTC],
                        rhs=ctr_t[:],
                        start=True,
                        stop=False,
                    )
                    for bi, g_t in enumerate((gA_t, gB_t)):
                        for k in range(KNN):
                            nc.tensor.matmul(
                                out=ps[:],
                                lhsT=w_t[:, k * OUTC : (k + 1) * OUTC],
                                rhs=g_t[:, k * NT : (k + 1) * NT],
                                start=False,
                                stop=(bi == 1 and k == KNN - 1),
                            )

                    o_t = outp.tile([OUTC, NT], mybir.dt.float32, tag="o")
                    nc.scalar.activation(
                        o_t[:], ps[:], mybir.ActivationFunctionType.Relu, bias=b_t[:]
                    )
                    nc.sync.dma_start(out=out_d[:, t * NT : (t + 1) * NT], in_=o_t[:])

    nc.compile()
    return nc


def _swizzle_table(tok):
    """(ranks*128, ELEM) token array -> (128, ranks*ELEM) SBUF preload layout."""
    ranks = tok.shape[0] // P
    return tok.reshape(ranks, P, ELEM).transpose(1, 0, 2).reshape(P, ranks * ELEM)


def _idx_swizzle(ix):
    """(TILES, NIDX) -> (TILES, 128, NIDX//16): [t, p, s] = ix[t, s*16 + p%16]."""
    a = ix.reshape(TILES, NIDX // 16, 16).transpose(0, 2, 1)  # (T, 16, cols)
    return np.broadcast_to(a[:, None], (TILES, 8, 16, NIDX // 16)).reshape(
        TILES, P, NIDX // 16
    )


def _prep_inputs(feats, aff_idx, locs, W, b):
    """Host-side prep of per-core input maps."""
    feats = np.asarray(feats)
    aff_idx = np.asarray(aff_idx)
    locs = np.asarray(locs)
    W = np.asarray(W, dtype=np.float32)
    b = np.asarray(b, dtype=np.float32)

    tables = []
    for g in range(N_GRAPHS):
        tokA = np.zeros((A_RANKS * P, ELEM), np.float16)
        tokA[:A_REAL, :C] = feats[g, :A_REAL]
        tokA[:A_REAL, C : C + 2] = locs[g, :A_REAL]
        tokB = np.zeros((B_TOK, ELEM), np.float16)
        tokB[:B_REAL, :C] = feats[g, A_REAL:]
        tokB[:B_REAL, C : C + 2] = locs[g, A_REAL:]
        tables.append(
            np.concatenate([_swizzle_table(tokA), _swizzle_table(tokB)], axis=1)
        )

    Wp = np.zeros((P, 10 * OUTC), np.float32)
    wloc_sum = np.zeros((2, OUTC), np.float32)
    for k in range(KNN):
        base = k * (C + 2)
        Wp[0:C, k * OUTC : (k + 1) * OUTC] = W[base + 2 : base + 2 + C]
        Wp[C : C + 2, k * OUTC : (k + 1) * OUTC] = W[base : base + 2] / (DIST + 1.0)
        wloc_sum += W[base : base + 2]
    Wp[0:C, KNN * OUTC :] = W[KNN * (C + 2) :]
    Wp[C : C + 2, KNN * OUTC :] = -wloc_sum / (DIST + 1.0)
    Wp = Wp.astype(np.float16)

    b_in = b.reshape(OUTC, 1).astype(np.float32)

    in_maps = []
    for core in range(8):
        g, h = core // 2, core % 2
        m0 = h * HALF
        nbr = aff_idx[g][m0 : m0 + HALF]                      # (HALF, 9)
        nbr = np.concatenate([nbr, np.zeros((NPAD - HALF, KNN), nbr.dtype)])
        flat = nbr.reshape(TILES, NT, KNN).transpose(0, 2, 1).reshape(TILES, NIDX)
        idxA = np.where(flat < A_REAL, flat, A_ZERO).astype(np.int16)
        idxB = np.where(flat >= A_REAL, flat - A_REAL, B_ZERO).astype(np.int16)
        idx_both = np.ascontiguousarray(
            np.concatenate([_idx_swizzle(idxA), _idx_swizzle(idxB)], axis=2)
        )
        ctrT = np.zeros((P, NPAD), np.float16)
        ctrT[0:C, :HALF] = feats[g, m0 : m0 + HALF].T
        ctrT[C : C + 2, :HALF] = locs[g, m0 : m0 + HALF].T
        in_maps.append(
            {"tab": tables[g], "ctr": ctrT, "idx": idx_both, "w": Wp, "b": b_in}
        )
    return in_maps


def kernel(feats, aff_idx, locs, W, b):
    from concourse.bass_utils import run_bass_kernel_spmd

    if "nc" not in _module_cache:
        _module_cache["nc"] = _build_module()
    nc = _module_cache["nc"]

    in_maps = _prep_inputs(feats, aff_idx, locs, W, b)
    res = run_bass_kernel_spmd(nc, in_maps, core_ids=list(range(8)))
    _module_cache["last_results"] = res

    out = np.empty((N_GRAPHS, M, OUTC), np.float32)
    for core in range(8):
        g, h = core // 2, core % 2
        out[g, h * HALF : (h + 1) * HALF] = res.results[core]["out"][:, :HALF].T
    return out
